# revision 12
# baseline (speedup 1.0000x reference)
"""Trainium2 Bass kernel for nn_Attention_62938450756123.

Reference computation (per batch b):
    oe[s, h] = out_e[s, b, 0:512] + out_e[s, b, 512:1024]      # bidirectional sum
    od[t, h] = out_d[t, b, :]
    S[s, t]  = sum_h oe[s, h] * od[t, h]
    p[s, t]  = exp(S[s, t])                                     # naive, no max-sub
    ctx[t,h] = (sum_s p[s, t] * oe[s, h]) / (sum_s p[s, t])
    out[t, b, h] = ctx[t, h]

Sharding: data-parallel over batch (bs=16) across 8 NeuronCores, 2 batches
per core, no collectives.

Per-core dataflow (all matmuls bf16 on TensorE, f32 PSUM accumulate):
  - GPSIMD (SWDGE) cast-loads f32->bf16 in ~1MB merged DMAs:
    out_e [128, 2s, 1024] (two s-tiles) and out_d [128, 4t, 512] (one
    t-chunk) per instruction - the SWDGE Q7 issues one DMA per ~650ns, so
    fewer/bigger transfers keep the head HBM-bound instead of issue-bound.
  - VectorE sums the out_e halves -> oe tiles bf16 [s128, h512].
  - h-major layouts are built ON TensorE: for each 128x128 block,
    psum[h, s'] = sum_s x[s, h] * I[s, s']  (normal matmul, identity moving,
    ~56ns warm).  Four h-chunks pack into one PSUM bank; one VectorE copy
    moves the bank to SBUF bf16: oeT_i [128p, 4hc, 128s],
    odT_chunk [128p, 4hc, 512t] (h = hc*128 + p).  DMA-xbar transposes are
    NOT used: Tile serializes them against every other DMA (HW-deadlock
    workaround), which makes the whole load stream ping-pong.
  - mm1 for t-chunks 0 AND 1 runs inside the load phase (lag-1 behind each
    oeT tile) so the PE stays saturated through the HBM-bound head; a ~6us
    dummy-matmul warmup un-throttles the HAM clock gate first.
  - mm1: psum_S[s128, t512] = sum_hc oeT_i.T @ odT ; exp on ScalarE -> P bf16
  - mm2: psum_ctx[t128, h512] += P_i.T @ oe_i ; psum_den[t128, 1] +=
    P_i.T @ ones   (same stationary weights, +25ns/pair measured)
  - normalize on VectorE (reciprocal + tensor_scalar), store via Sync HWDGE.

Buffers are allocated per-s-tile (separate Tile objects) so dependency
tracking stays precise.
"""

import ml_dtypes
import numpy as np

import concourse.bass as bass
import concourse.tile as tile
from concourse import bacc, mybir
from concourse.bass_utils import run_bass_kernel_spmd

SL, TL, BS, H = 2048, 2048, 16, 512
NCORES = 8
BPC = BS // NCORES  # batches per core

F32 = mybir.dt.float32
BF16 = mybir.dt.bfloat16

NS = SL // 128        # 16 s-tiles
NH = H // 128         # 4 h-chunks
TCHUNK = 512          # t-chunk (one PSUM bank of f32)
NTC = TL // TCHUNK    # 4 t-chunks
TPC = TCHUNK // 128   # 4 t-tiles per chunk
HEAD_CHUNKS = 2       # t-chunks whose mm1 runs inside the load phase


def build():
    nc = bacc.Bacc("TRN2", target_bir_lowering=False, debug=False,
                   num_devices=NCORES)
    out_e = nc.dram_tensor("out_e", [SL, BPC, 2 * H], F32,
                           kind="ExternalInput").ap()
    out_d = nc.dram_tensor("out_d", [TL, BPC, H], F32,
                           kind="ExternalInput").ap()
    ident = nc.dram_tensor("ident", [128, 128], BF16,
                           kind="ExternalInput").ap()
    out = nc.dram_tensor("out", [TL, BPC, H], F32,
                         kind="ExternalOutput").ap()

    exp = mybir.ActivationFunctionType.Exp

    with tile.TileContext(nc) as tc:
        with (
            tc.tile_pool(name="consts", bufs=1) as consts,
            tc.tile_pool(name="stage_e", bufs=4) as stage_e_pool,
            tc.tile_pool(name="stage_d", bufs=4) as stage_d_pool,
            tc.tile_pool(name="oenat", bufs=2 * NS) as oenat_pool,
            tc.tile_pool(name="oet", bufs=2 * NS) as oet_pool,
            tc.tile_pool(name="odt", bufs=2 * NTC) as odt_pool,
            tc.tile_pool(name="pbuf", bufs=3 * NS) as p_pool,
            tc.tile_pool(name="osb", bufs=3) as osb_pool,
            tc.tile_pool(name="small", bufs=4) as small_pool,
            tc.tile_pool(name="psS", bufs=3, space="PSUM") as psS_pool,
            tc.tile_pool(name="psC", bufs=2, space="PSUM") as psC_pool,
            tc.tile_pool(name="psD", bufs=1, space="PSUM") as psD_pool,
            tc.tile_pool(name="ptr", bufs=2, space="PSUM") as ptr_pool,
        ):
            ones = consts.tile([128, 1], BF16, tag="ones")
            nc.vector.memset(ones, 1.0)
            idt = consts.tile([128, 128], BF16, tag="idt")
            nc.sync.dma_start(idt, ident)

            # ~6us of dummy matmuls at program start: HAM un-throttles the
            # PE clock (1.2 -> 2.4 GHz) after ~3.4us of sustained activity,
            # so the load-phase work runs warm.  Overlaps the first loads.
            warm = consts.tile([128, TCHUNK], BF16, tag="warm")
            nc.vector.memset(warm, 0.25)
            wt = ptr_pool.tile([128, TCHUNK], F32, tag="ptr")
            for _ in range(28):
                nc.tensor.matmul(wt, warm[:, 0:128], warm,
                                 start=True, stop=True)

            def transpose_tiles(src, dst):
                """src [128, NH*128] bf16 -> dst [128, NH, 128] with
                dst[p, c, j] = src[j, c*128 + p], via NH identity matmuls
                packed into one PSUM bank + one DVE copy."""
                pt = ptr_pool.tile([128, NH * 128], F32, tag="ptr")
                for c in range(NH):
                    nc.tensor.matmul(pt[:, c * 128:(c + 1) * 128],
                                     src[:, c * 128:(c + 1) * 128], idt,
                                     start=True, stop=True)
                nc.vector.tensor_copy(dst, pt)

            for b in range(BPC):
                # per-s-tile / per-chunk buffers for this batch
                oe_tiles = []    # [128, H] bf16, natural layout
                oeT_tiles = []   # [128, NH, 128] bf16, h-major
                odT_chunks = []  # [128, NH, TCHUNK] bf16, h-major
                P_tiles = {tci: [] for tci in range(NTC)}

                def load_d(ci):
                    # one t-chunk (4 t-tiles) per merged SWDGE cast-load
                    odc = odt_pool.tile([128, NH, TCHUNK], BF16, tag="odT",
                                        name=f"odT_{b}_{ci}")
                    odT_chunks.append(odc)
                    sd = stage_d_pool.tile([128, TPC, H], BF16, tag="sd",
                                           name=f"sd_{b}_{ci}")
                    src = out_d[ci * TCHUNK:(ci + 1) * TCHUNK, b, :]
                    nc.gpsimd.dma_start(
                        sd, src.rearrange("(k p) h -> p k h", p=128))
                    for k in range(TPC):
                        transpose_tiles(sd[:, k, :],
                                        odc[:, :, k * 128:(k + 1) * 128])

                def load_e(j):
                    # two s-tiles (both halves) per merged SWDGE cast-load
                    st = stage_e_pool.tile([128, 2, 2 * H], BF16, tag="st",
                                           name=f"st_{b}_{j}")
                    src = out_e[j * 256:(j + 1) * 256, b, :]
                    nc.gpsimd.dma_start(
                        st, src.rearrange("(k p) h -> p k h", p=128))
                    for k in range(2):
                        oe = oenat_pool.tile([128, H], BF16, tag="oe",
                                             name=f"oe_{b}_{2 * j + k}")
                        oeT = oet_pool.tile([128, NH, 128], BF16, tag="oeT",
                                            name=f"oeT_{b}_{2 * j + k}")
                        oe_tiles.append(oe)
                        oeT_tiles.append(oeT)
                        nc.vector.tensor_add(oe, st[:, k, 0:H],
                                             st[:, k, H:2 * H])
                        transpose_tiles(oe, oeT)

                def mm1(tci, i):
                    psS = psS_pool.tile([128, TCHUNK], F32, tag="psS")
                    for c in range(NH):
                        nc.tensor.matmul(
                            psS,
                            oeT_tiles[i][:, c, :],
                            odT_chunks[tci][:, c, :],
                            start=(c == 0), stop=(c == NH - 1))
                    P = p_pool.tile([128, TCHUNK], BF16, tag="P",
                                    name=f"P_{b}_{tci}_{i}")
                    P_tiles[tci].append(P)
                    nc.scalar.activation(P, psS, exp)

                def mm2(tci):
                    for tt in range(TPC):
                        psC = psC_pool.tile([128, H], F32, tag="psC")
                        psD = psD_pool.tile([128, 1], F32, tag="psD")
                        for i in range(NS):
                            lhsT = P_tiles[tci][i][:, tt * 128:(tt + 1) * 128]
                            nc.tensor.matmul(psC, lhsT, oe_tiles[i],
                                             start=(i == 0), stop=(i == NS - 1))
                            nc.tensor.matmul(psD, lhsT, ones,
                                             start=(i == 0), stop=(i == NS - 1))
                        rc = small_pool.tile([128, 1], F32, tag="rc")
                        nc.vector.reciprocal(rc, psD)
                        ob = osb_pool.tile([128, H], F32, tag="ob")
                        nc.vector.tensor_scalar(ob, psC, rc, None,
                                                mybir.AluOpType.mult)
                        t0 = tci * TCHUNK + tt * 128
                        nc.sync.dma_start(out[t0:t0 + 128, b, :], ob)

                # Load phase: stream loads; transposes follow each arrival;
                # mm1 for the first HEAD_CHUNKS t-chunks trails one e-load
                # behind (hides the PE->DVE->PE oeT round trip).
                for ci in range(HEAD_CHUNKS):
                    load_d(ci)
                for j in range(NS // 2):
                    load_e(j)
                    if j < NTC - HEAD_CHUNKS:
                        load_d(HEAD_CHUNKS + j)
                    if j >= 1:
                        for s in (2 * (j - 1), 2 * j - 1):
                            for tci in range(HEAD_CHUNKS):
                                mm1(tci, s)
                for s in (NS - 2, NS - 1):
                    for tci in range(HEAD_CHUNKS):
                        mm1(tci, s)

                # Steady phase: alternate remaining mm1 chunks with mm2 so
                # the PE stream never waits on exp.
                for tci in range(HEAD_CHUNKS, NTC):
                    for i in range(NS):
                        mm1(tci, i)
                    mm2(tci - HEAD_CHUNKS)
                for tci in range(NTC - HEAD_CHUNKS, NTC):
                    mm2(tci)

    nc.compile()
    return nc


_nc = None
last_result = None
_IDENT = np.eye(128).astype(ml_dtypes.bfloat16)


def kernel(in_e=None, out_e=None, out_d=None, _trace=False, **_unused):
    global _nc, last_result
    if _nc is None:
        _nc = build()
    out_e = np.asarray(out_e, dtype=np.float32)
    out_d = np.asarray(out_d, dtype=np.float32)
    in_maps = []
    for c in range(NCORES):
        sl = slice(c * BPC, (c + 1) * BPC)
        in_maps.append({
            "out_e": np.ascontiguousarray(out_e[:, sl, :]),
            "out_d": np.ascontiguousarray(out_d[:, sl, :]),
            "ident": _IDENT,
        })
    last_result = run_bass_kernel_spmd(_nc, in_maps,
                                       core_ids=list(range(NCORES)),
                                       trace=_trace)
    return np.concatenate(
        [np.asarray(last_result.results[c]["out"]) for c in range(NCORES)],
        axis=1).astype(np.float32)


# revision 13
# speedup vs baseline: 1.2445x; 1.2445x over previous
"""Trainium2 Bass kernel for nn_Attention_62938450756123.

Reference computation (per batch b):
    oe[s, h] = out_e[s, b, 0:512] + out_e[s, b, 512:1024]      # bidirectional sum
    od[t, h] = out_d[t, b, :]
    S[s, t]  = sum_h oe[s, h] * od[t, h]
    p[s, t]  = exp(S[s, t])                                     # naive, no max-sub
    ctx[t,h] = (sum_s p[s, t] * oe[s, h]) / (sum_s p[s, t])
    out[t, b, h] = ctx[t, h]

Sharding: data-parallel over batch (bs=16) across 8 NeuronCores, 2 batches
per core, no collectives.

Per-core dataflow:
  - GPSIMD (SWDGE) cast-loads f32->bf16: out_e halves + out_d tiles.
  - VectorE sums the out_e halves -> oe tiles bf16 [s128, h512] (mm2 rhs).
  - h-major layouts for mm1 are built ON TensorE: for each 128x128 block,
    psum[h, s'] = sum_s x[s, h] * (SCALE * I[s, s'])  (normal matmul,
    scaled identity moving, ~56ns warm).  Four h-chunks pack into one PSUM
    bank; one VectorE copy casts the bank to fp8e4m3 SBUF:
    oeT_i [128p, 4hc, 128s], odT_chunk [128p, 4hc, 512t], h = hc*128 + p,
    values pre-scaled by SCALE=32 to sit in fp8's normal range.
    (DMA-xbar transposes are NOT used: Tile serializes them against every
    other DMA - HW-deadlock workaround - which ping-pongs the load stream.)
  - mm1 runs in fp8 with perf_mode=DoubleRow (2 fp8 weights/PE cell):
    psum_S[s128, t512] accumulates over 2 k-tiles of [128p x 2ko] = 256,
    at ~2x bf16 matmul rate.  ScalarE exp applies scale=1/SCALE^2 to undo
    the identity pre-scaling: P = exp(S_psum / 1024) in bf16.
    Softmax output error stays ~1e-3: S absolute error ~1e-3 from fp8
    inputs, and the near-uniform softmax averages 2048 terms.
  - mm2 stays bf16: psum_ctx[t128, h512] += P_i.T @ oe_i ;
    psum_den[t128, 1] += P_i.T @ ones (same stationary weights, +25ns/pair)
  - normalize on VectorE (reciprocal + tensor_scalar), store via Sync HWDGE.
  - ~6us dummy-matmul warmup un-throttles the HAM PE clock gate before the
    load phase; chunk-0 mm1 trails the per-tile transposes by one s-tile.

Buffers are allocated per-s-tile (separate Tile objects) so dependency
tracking stays precise.
"""

import ml_dtypes
import numpy as np

import concourse.bass as bass
import concourse.tile as tile
from concourse import bacc, mybir
from concourse.bass_utils import run_bass_kernel_spmd

SL, TL, BS, H = 2048, 2048, 16, 512
NCORES = 8
BPC = BS // NCORES  # batches per core

F32 = mybir.dt.float32
BF16 = mybir.dt.bfloat16
FP8 = mybir.dt.float8e4

NS = SL // 128        # 16 s-tiles
NH = H // 128         # 4 h-chunks
TCHUNK = 512          # t-chunk (one PSUM bank of f32)
NTC = TL // TCHUNK    # 4 t-chunks
TPC = TCHUNK // 128   # 4 t-tiles per chunk
SCALE = 32.0          # fp8 pre-scale (folded into the transpose identity)


def build():
    nc = bacc.Bacc("TRN2", target_bir_lowering=False, debug=False,
                   num_devices=NCORES)
    out_e = nc.dram_tensor("out_e", [SL, BPC, 2 * H], F32,
                           kind="ExternalInput").ap()
    out_d = nc.dram_tensor("out_d", [TL, BPC, H], F32,
                           kind="ExternalInput").ap()
    ident = nc.dram_tensor("ident", [128, 128], BF16,
                           kind="ExternalInput").ap()
    out = nc.dram_tensor("out", [TL, BPC, H], F32,
                         kind="ExternalOutput").ap()

    exp = mybir.ActivationFunctionType.Exp
    dr = mybir.MatmulPerfMode.DoubleRow

    with tile.TileContext(nc) as tc:
        with (
            tc.tile_pool(name="consts", bufs=1) as consts,
            tc.tile_pool(name="stage_e", bufs=8) as stage_e_pool,
            tc.tile_pool(name="odnat", bufs=8) as odnat_pool,
            tc.tile_pool(name="oenat", bufs=2 * NS) as oenat_pool,
            tc.tile_pool(name="oet", bufs=2 * NS) as oet_pool,
            tc.tile_pool(name="odt", bufs=2 * NTC) as odt_pool,
            tc.tile_pool(name="pbuf", bufs=2 * NS) as p_pool,
            tc.tile_pool(name="osb", bufs=3) as osb_pool,
            tc.tile_pool(name="small", bufs=4) as small_pool,
            tc.tile_pool(name="psS", bufs=3, space="PSUM") as psS_pool,
            tc.tile_pool(name="psC", bufs=2, space="PSUM") as psC_pool,
            tc.tile_pool(name="psD", bufs=1, space="PSUM") as psD_pool,
            tc.tile_pool(name="ptr", bufs=2, space="PSUM") as ptr_pool,
        ):
            ones = consts.tile([128, 1], BF16, tag="ones")
            nc.vector.memset(ones, 1.0)
            idt = consts.tile([128, 128], BF16, tag="idt")
            nc.sync.dma_start(idt, ident)

            # HAM warmup: un-throttle the PE clock before the load phase.
            warm = consts.tile([128, TCHUNK], BF16, tag="warm")
            nc.vector.memset(warm, 0.25)
            wt = ptr_pool.tile([128, TCHUNK], F32, tag="ptr")
            for _ in range(28):
                nc.tensor.matmul(wt, warm[:, 0:128], warm,
                                 start=True, stop=True)

            def transpose_tiles(src, dst):
                """src [128, NH*128] bf16 -> dst [128, NH, 128] fp8 with
                dst[p, c, j] = SCALE * src[j, c*128 + p], via NH identity
                matmuls packed into one PSUM bank + one DVE copy-cast."""
                pt = ptr_pool.tile([128, NH * 128], F32, tag="ptr")
                for c in range(NH):
                    nc.tensor.matmul(pt[:, c * 128:(c + 1) * 128],
                                     src[:, c * 128:(c + 1) * 128], idt,
                                     start=True, stop=True)
                nc.vector.tensor_copy(dst, pt)

            for b in range(BPC):
                # per-s-tile buffers for this batch
                oe_tiles = []    # [128, H] bf16, natural layout (mm2 rhs)
                oeT_tiles = []   # [128, NH, 128] fp8, h-major, x SCALE
                odT_chunks = []  # [128, NH, TCHUNK] fp8, h-major, x SCALE

                def load_d(i):
                    # SWDGE cast-load f32 -> bf16; 4 od t-tiles feed one
                    # odT chunk tile.
                    ci, k = divmod(i, TPC)
                    if k == 0:
                        odc = odt_pool.tile([128, NH, TCHUNK], FP8, tag="odT",
                                            name=f"odT_{b}_{ci}")
                        odT_chunks.append(odc)
                    odc = odT_chunks[ci]
                    sd = odnat_pool.tile([128, H], BF16, tag="od",
                                         name=f"od_{b}_{i}")
                    nc.gpsimd.dma_start(sd, out_d[i * 128:(i + 1) * 128, b, :])
                    transpose_tiles(sd, odc[:, :, k * 128:(k + 1) * 128])

                def load_e(i):
                    oe = oenat_pool.tile([128, H], BF16, tag="oe",
                                         name=f"oe_{b}_{i}")
                    oeT = oet_pool.tile([128, NH, 128], FP8, tag="oeT",
                                        name=f"oeT_{b}_{i}")
                    oe_tiles.append(oe)
                    oeT_tiles.append(oeT)
                    h1 = stage_e_pool.tile([128, H], BF16, tag="st",
                                           name=f"h1_{b}_{i}")
                    nc.gpsimd.dma_start(oe, out_e[i * 128:(i + 1) * 128, b, 0:H])
                    nc.gpsimd.dma_start(h1, out_e[i * 128:(i + 1) * 128, b, H:2 * H])
                    nc.vector.tensor_add(oe, oe, h1)
                    transpose_tiles(oe, oeT)

                def mm1(tci, i, P_tiles):
                    psS = psS_pool.tile([128, TCHUNK], F32, tag="psS")
                    for c2 in range(NH // 2):
                        nc.tensor.matmul(
                            psS,
                            oeT_tiles[i][:, 2 * c2:2 * c2 + 2, :],
                            odT_chunks[tci][:, 2 * c2:2 * c2 + 2, :],
                            start=(c2 == 0), stop=(c2 == NH // 2 - 1),
                            perf_mode=dr)
                    P = p_pool.tile([128, TCHUNK], BF16, tag="P",
                                    name=f"P_{b}_{tci}_{i}")
                    P_tiles.append(P)
                    # undo the SCALE^2 from the pre-scaled transposes
                    nc.scalar.activation(P, psS, exp,
                                         scale=1.0 / (SCALE * SCALE))

                # Pipeline batch preprocessing with chunk-0 mm1: transposes
                # for s-tile i are followed by mm1 on s-tile i-1 (one tile of
                # lag hides the PE->DVE->PE round trip through oeT).
                P0_tiles = []
                for i in range(TPC):
                    load_d(i)
                for i in range(NS):
                    load_e(i)
                    if TPC + i < NS:
                        load_d(TPC + i)
                    if i >= 1:
                        mm1(0, i - 1, P0_tiles)
                mm1(0, NS - 1, P0_tiles)

                for tci in range(NTC):
                    P_tiles = P0_tiles if tci == 0 else []
                    if tci > 0:
                        for i in range(NS):
                            mm1(tci, i, P_tiles)
                    for tt in range(TPC):
                        psC = psC_pool.tile([128, H], F32, tag="psC")
                        psD = psD_pool.tile([128, 1], F32, tag="psD")
                        for i in range(NS):
                            lhsT = P_tiles[i][:, tt * 128:(tt + 1) * 128]
                            nc.tensor.matmul(psC, lhsT, oe_tiles[i],
                                             start=(i == 0), stop=(i == NS - 1))
                            nc.tensor.matmul(psD, lhsT, ones,
                                             start=(i == 0), stop=(i == NS - 1))
                        rc = small_pool.tile([128, 1], F32, tag="rc")
                        nc.vector.reciprocal(rc, psD)
                        ob = osb_pool.tile([128, H], F32, tag="ob")
                        nc.vector.tensor_scalar(ob, psC, rc, None,
                                                mybir.AluOpType.mult)
                        t0 = tci * TCHUNK + tt * 128
                        nc.sync.dma_start(out[t0:t0 + 128, b, :], ob)

    nc.compile()
    return nc


_nc = None
last_result = None
_IDENT = (np.eye(128) * SCALE).astype(ml_dtypes.bfloat16)


def kernel(in_e=None, out_e=None, out_d=None, _trace=False, **_unused):
    global _nc, last_result
    if _nc is None:
        _nc = build()
    out_e = np.asarray(out_e, dtype=np.float32)
    out_d = np.asarray(out_d, dtype=np.float32)
    in_maps = []
    for c in range(NCORES):
        sl = slice(c * BPC, (c + 1) * BPC)
        in_maps.append({
            "out_e": np.ascontiguousarray(out_e[:, sl, :]),
            "out_d": np.ascontiguousarray(out_d[:, sl, :]),
            "ident": _IDENT,
        })
    last_result = run_bass_kernel_spmd(_nc, in_maps,
                                       core_ids=list(range(NCORES)),
                                       trace=_trace)
    return np.concatenate(
        [np.asarray(last_result.results[c]["out"]) for c in range(NCORES)],
        axis=1).astype(np.float32)


# revision 15
# speedup vs baseline: 1.3039x; 1.0477x over previous
"""Trainium2 Bass kernel for nn_Attention_62938450756123.

Reference computation (per batch b):
    oe[s, h] = out_e[s, b, 0:512] + out_e[s, b, 512:1024]      # bidirectional sum
    od[t, h] = out_d[t, b, :]
    S[s, t]  = sum_h oe[s, h] * od[t, h]
    p[s, t]  = exp(S[s, t])                                     # naive, no max-sub
    ctx[t,h] = (sum_s p[s, t] * oe[s, h]) / (sum_s p[s, t])
    out[t, b, h] = ctx[t, h]

Sharding: data-parallel over batch (bs=16) across 8 NeuronCores, 2 batches
per core, no collectives.

Per-core dataflow:
  - GPSIMD (SWDGE) cast-loads f32->bf16: out_e halves + out_d tiles.
  - VectorE sums the out_e halves -> oe tiles bf16 [s128, h512] (mm2 rhs).
  - h-major layouts for mm1 are built ON TensorE: for each 128x128 block,
    psum[h, s'] = sum_s x[s, h] * (SCALE * I[s, s'])  (normal matmul,
    scaled identity moving, ~56ns warm).  Four h-chunks pack into one PSUM
    bank; one VectorE copy casts the bank to fp8e4m3 SBUF:
    oeT_i [128p, 4hc, 128s], odT_chunk [128p, 4hc, 512t], h = hc*128 + p,
    values pre-scaled by SCALE=32 to sit in fp8's normal range.
    (DMA-xbar transposes are NOT used: Tile serializes them against every
    other DMA - HW-deadlock workaround - which ping-pongs the load stream.)
  - mm1 runs in fp8 with perf_mode=DoubleRow (2 fp8 weights/PE cell):
    psum_S[s128, t512] accumulates over 2 k-tiles of [128p x 2ko] = 256,
    at ~2x bf16 matmul rate.  ScalarE exp applies scale=1/SCALE^2 to undo
    the identity pre-scaling: P = exp(S_psum / 1024) in bf16.
    Softmax output error stays ~1e-3: S absolute error ~1e-3 from fp8
    inputs, and the near-uniform softmax averages 2048 terms.
  - mm2 stays bf16: psum_ctx[t128, h512] += P_i.T @ oe_i ;
    psum_den[t128, 1] += P_i.T @ ones (same stationary weights, +25ns/pair)
  - normalize on VectorE (reciprocal + tensor_scalar), store via Sync HWDGE.
  - ~6us dummy-matmul warmup un-throttles the HAM PE clock gate before the
    load phase; chunk-0 mm1 trails the per-tile transposes by one s-tile.

Buffers are allocated per-s-tile (separate Tile objects) so dependency
tracking stays precise.
"""

import ml_dtypes
import numpy as np

import concourse.bass as bass
import concourse.tile as tile
from concourse import bacc, mybir
from concourse.bass_utils import run_bass_kernel_spmd

SL, TL, BS, H = 2048, 2048, 16, 512
NCORES = 8
BPC = BS // NCORES  # batches per core

F32 = mybir.dt.float32
BF16 = mybir.dt.bfloat16
FP8 = mybir.dt.float8e4

NS = SL // 128        # 16 s-tiles
NH = H // 128         # 4 h-chunks
TCHUNK = 512          # t-chunk (one PSUM bank of f32)
NTC = TL // TCHUNK    # 4 t-chunks
TPC = TCHUNK // 128   # 4 t-tiles per chunk
SCALE = 32.0          # fp8 pre-scale (folded into the transpose identity)


def build():
    nc = bacc.Bacc("TRN2", target_bir_lowering=False, debug=False,
                   num_devices=NCORES)
    out_e = nc.dram_tensor("out_e", [SL, BPC, 2 * H], F32,
                           kind="ExternalInput").ap()
    out_d = nc.dram_tensor("out_d", [TL, BPC, H], F32,
                           kind="ExternalInput").ap()
    ident = nc.dram_tensor("ident", [128, 128], BF16,
                           kind="ExternalInput").ap()
    out = nc.dram_tensor("out", [TL, BPC, H], F32,
                         kind="ExternalOutput").ap()

    exp = mybir.ActivationFunctionType.Exp
    dr = mybir.MatmulPerfMode.DoubleRow

    with tile.TileContext(nc) as tc:
        with (
            tc.tile_pool(name="consts", bufs=1) as consts,
            tc.tile_pool(name="stage_e", bufs=4) as stage_e_pool,
            tc.tile_pool(name="stage_d", bufs=4) as stage_d_pool,
            tc.tile_pool(name="oenat", bufs=2 * NS) as oenat_pool,
            tc.tile_pool(name="oet", bufs=2 * NS) as oet_pool,
            tc.tile_pool(name="odt", bufs=2 * NTC) as odt_pool,
            tc.tile_pool(name="pbuf", bufs=3 * NS) as p_pool,
            tc.tile_pool(name="osb", bufs=3) as osb_pool,
            tc.tile_pool(name="small", bufs=4) as small_pool,
            tc.tile_pool(name="psS", bufs=3, space="PSUM") as psS_pool,
            tc.tile_pool(name="psC", bufs=2, space="PSUM") as psC_pool,
            tc.tile_pool(name="psD", bufs=1, space="PSUM") as psD_pool,
            tc.tile_pool(name="ptr", bufs=2, space="PSUM") as ptr_pool,
        ):
            ones = consts.tile([128, 1], BF16, tag="ones")
            nc.vector.memset(ones, 1.0)
            idt = consts.tile([128, 128], BF16, tag="idt")
            nc.sync.dma_start(idt, ident)

            # HAM warmup: un-throttle the PE clock before the load phase.
            warm = consts.tile([128, TCHUNK], BF16, tag="warm")
            nc.vector.memset(warm, 0.25)
            wt = ptr_pool.tile([128, TCHUNK], F32, tag="ptr")
            for _ in range(28):
                nc.tensor.matmul(wt, warm[:, 0:128], warm,
                                 start=True, stop=True)

            def transpose_tiles(src, dst):
                """src [128, NH*128] bf16 -> dst [128, NH, 128] fp8 with
                dst[p, c, j] = SCALE * src[j, c*128 + p], via NH identity
                matmuls packed into one PSUM bank + one DVE copy-cast."""
                pt = ptr_pool.tile([128, NH * 128], F32, tag="ptr")
                for c in range(NH):
                    nc.tensor.matmul(pt[:, c * 128:(c + 1) * 128],
                                     src[:, c * 128:(c + 1) * 128], idt,
                                     start=True, stop=True)
                nc.vector.tensor_copy(dst, pt)

            HEAD_CHUNKS = 2
            for b in range(BPC):
                # per-s-tile buffers for this batch
                oe_tiles = []    # [128, H] bf16, natural layout (mm2 rhs)
                oeT_tiles = []   # [128, NH, 128] fp8, h-major, x SCALE
                odT_chunks = []  # [128, NH, TCHUNK] fp8, h-major, x SCALE
                P_tiles = {tci: [] for tci in range(NTC)}

                def load_d(ci):
                    # one t-chunk (4 t-tiles) per merged SWDGE cast-load
                    odc = odt_pool.tile([128, NH, TCHUNK], FP8, tag="odT",
                                        name=f"odT_{b}_{ci}")
                    odT_chunks.append(odc)
                    sd = stage_d_pool.tile([128, TPC, H], BF16, tag="sd",
                                           name=f"sd_{b}_{ci}")
                    src = out_d[ci * TCHUNK:(ci + 1) * TCHUNK, b, :]
                    nc.gpsimd.dma_start(
                        sd, src.rearrange("(k p) h -> p k h", p=128))
                    for k in range(TPC):
                        transpose_tiles(sd[:, k, :],
                                        odc[:, :, k * 128:(k + 1) * 128])

                def load_e(j):
                    # two s-tiles (both halves) per merged SWDGE cast-load
                    st = stage_e_pool.tile([128, 2, 2 * H], BF16, tag="st",
                                           name=f"st_{b}_{j}")
                    src = out_e[j * 256:(j + 1) * 256, b, :]
                    nc.gpsimd.dma_start(
                        st, src.rearrange("(k p) h -> p k h", p=128))
                    for k in range(2):
                        oe = oenat_pool.tile([128, H], BF16, tag="oe",
                                             name=f"oe_{b}_{2 * j + k}")
                        oeT = oet_pool.tile([128, NH, 128], FP8, tag="oeT",
                                            name=f"oeT_{b}_{2 * j + k}")
                        oe_tiles.append(oe)
                        oeT_tiles.append(oeT)
                        nc.vector.tensor_add(oe, st[:, k, 0:H],
                                             st[:, k, H:2 * H])
                        transpose_tiles(oe, oeT)

                def mm1(tci, i):
                    psS = psS_pool.tile([128, TCHUNK], F32, tag="psS")
                    for c2 in range(NH // 2):
                        nc.tensor.matmul(
                            psS,
                            oeT_tiles[i][:, 2 * c2:2 * c2 + 2, :],
                            odT_chunks[tci][:, 2 * c2:2 * c2 + 2, :],
                            start=(c2 == 0), stop=(c2 == NH // 2 - 1),
                            perf_mode=dr)
                    P = p_pool.tile([128, TCHUNK], BF16, tag="P",
                                    name=f"P_{b}_{tci}_{i}")
                    P_tiles[tci].append(P)
                    # undo the SCALE^2 from the pre-scaled transposes
                    nc.scalar.activation(P, psS, exp,
                                         scale=1.0 / (SCALE * SCALE))

                def mm2(tci):
                    for tt in range(TPC):
                        psC = psC_pool.tile([128, H], F32, tag="psC")
                        psD = psD_pool.tile([128, 1], F32, tag="psD")
                        for i in range(NS):
                            lhsT = P_tiles[tci][i][:, tt * 128:(tt + 1) * 128]
                            nc.tensor.matmul(psC, lhsT, oe_tiles[i],
                                             start=(i == 0), stop=(i == NS - 1))
                            nc.tensor.matmul(psD, lhsT, ones,
                                             start=(i == 0), stop=(i == NS - 1))
                        rc = small_pool.tile([128, 1], F32, tag="rc")
                        nc.vector.reciprocal(rc, psD)
                        ob = osb_pool.tile([128, H], F32, tag="ob")
                        nc.vector.tensor_scalar(ob, psC, rc, None,
                                                mybir.AluOpType.mult)
                        t0 = tci * TCHUNK + tt * 128
                        nc.sync.dma_start(out[t0:t0 + 128, b, :], ob)

                # Load phase: merged loads stream; transposes follow each
                # arrival; mm1 for the first HEAD_CHUNKS t-chunks trails one
                # e-load behind (hides the PE->DVE->PE oeT round trip).
                for ci in range(HEAD_CHUNKS):
                    load_d(ci)
                for j in range(NS // 2):
                    load_e(j)
                    if j < NTC - HEAD_CHUNKS:
                        load_d(HEAD_CHUNKS + j)
                    if j >= 1:
                        for s in (2 * (j - 1), 2 * j - 1):
                            for tci in range(HEAD_CHUNKS):
                                mm1(tci, s)
                for s in (NS - 2, NS - 1):
                    for tci in range(HEAD_CHUNKS):
                        mm1(tci, s)

                # Steady phase: alternate remaining mm1 chunks with mm2.
                for tci in range(HEAD_CHUNKS, NTC):
                    for i in range(NS):
                        mm1(tci, i)
                    mm2(tci - HEAD_CHUNKS)
                for tci in range(NTC - HEAD_CHUNKS, NTC):
                    mm2(tci)

    nc.compile()
    return nc


_nc = None
last_result = None
_IDENT = (np.eye(128) * SCALE).astype(ml_dtypes.bfloat16)


def kernel(in_e=None, out_e=None, out_d=None, _trace=False, **_unused):
    global _nc, last_result
    if _nc is None:
        _nc = build()
    out_e = np.asarray(out_e, dtype=np.float32)
    out_d = np.asarray(out_d, dtype=np.float32)
    in_maps = []
    for c in range(NCORES):
        sl = slice(c * BPC, (c + 1) * BPC)
        in_maps.append({
            "out_e": np.ascontiguousarray(out_e[:, sl, :]),
            "out_d": np.ascontiguousarray(out_d[:, sl, :]),
            "ident": _IDENT,
        })
    last_result = run_bass_kernel_spmd(_nc, in_maps,
                                       core_ids=list(range(NCORES)),
                                       trace=_trace)
    return np.concatenate(
        [np.asarray(last_result.results[c]["out"]) for c in range(NCORES)],
        axis=1).astype(np.float32)


# revision 21
# speedup vs baseline: 1.3721x; 1.0523x over previous
"""Trainium2 Bass kernel for nn_Attention_62938450756123.

Reference computation (per batch b):
    oe[s, h] = out_e[s, b, 0:512] + out_e[s, b, 512:1024]      # bidirectional sum
    od[t, h] = out_d[t, b, :]
    S[s, t]  = sum_h oe[s, h] * od[t, h]
    p[s, t]  = exp(S[s, t])                                     # naive, no max-sub
    ctx[t,h] = (sum_s p[s, t] * oe[s, h]) / (sum_s p[s, t])
    out[t, b, h] = ctx[t, h]

Sharding: data-parallel over batch (bs=16) across 8 NeuronCores, 2 batches
per core, no collectives.

Per-core dataflow:
  - GPSIMD (SWDGE) cast-loads f32->bf16: out_e halves + out_d tiles.
  - VectorE sums the out_e halves -> oe tiles bf16 [s128, h512] (mm2 rhs).
  - h-major layouts for mm1 are built ON TensorE: for each 128x128 block,
    psum[h, s'] = sum_s x[s, h] * (SCALE * I[s, s'])  (normal matmul,
    scaled identity moving, ~56ns warm).  Four h-chunks pack into one PSUM
    bank; one VectorE copy casts the bank to fp8e4m3 SBUF:
    oeT_i [128p, 4hc, 128s], odT_chunk [128p, 4hc, 512t], h = hc*128 + p,
    values pre-scaled by SCALE=32 to sit in fp8's normal range.
    (DMA-xbar transposes are NOT used: Tile serializes them against every
    other DMA - HW-deadlock workaround - which ping-pongs the load stream.)
  - mm1 runs in fp8 with perf_mode=DoubleRow (2 fp8 weights/PE cell):
    psum_S[s128, t512] accumulates over 2 k-tiles of [128p x 2ko] = 256,
    at ~2x bf16 matmul rate.  ScalarE exp applies scale=1/SCALE^2 to undo
    the identity pre-scaling: P = exp(S_psum / 1024) in bf16.
    Softmax output error stays ~1e-3: S absolute error ~1e-3 from fp8
    inputs, and the near-uniform softmax averages 2048 terms.
  - mm2 stays bf16: psum_ctx[t128, h512] += P_i.T @ oe_i ;
    psum_den[t128, 1] += P_i.T @ ones (same stationary weights, +25ns/pair)
  - normalize on VectorE (reciprocal + tensor_scalar), store via Sync HWDGE.
  - ~6us dummy-matmul warmup un-throttles the HAM PE clock gate before the
    load phase; chunk-0 mm1 trails the per-tile transposes by one s-tile.

Buffers are allocated per-s-tile (separate Tile objects) so dependency
tracking stays precise.
"""

import ml_dtypes
import numpy as np

import concourse.bass as bass
import concourse.tile as tile
from concourse import bacc, mybir
from concourse.bass_utils import run_bass_kernel_spmd

SL, TL, BS, H = 2048, 2048, 16, 512
NCORES = 8
BPC = BS // NCORES  # batches per core

F32 = mybir.dt.float32
BF16 = mybir.dt.bfloat16
FP8 = mybir.dt.float8e4

NS = SL // 128        # 16 s-tiles
NH = H // 128         # 4 h-chunks
TCHUNK = 512          # t-chunk (one PSUM bank of f32)
NTC = TL // TCHUNK    # 4 t-chunks
TPC = TCHUNK // 128   # 4 t-tiles per chunk
SCALE = 32.0          # fp8 pre-scale (folded into the transpose identity)
DSCALE = 16.0         # fp8 pre-scale for d = p - 1 (|d| <~ 0.06 -> ~1)


def build():
    nc = bacc.Bacc("TRN2", target_bir_lowering=False, debug=False,
                   num_devices=NCORES)
    out_e = nc.dram_tensor("out_e", [SL, BPC, 2 * H], F32,
                           kind="ExternalInput").ap()
    out_d = nc.dram_tensor("out_d", [TL, BPC, H], F32,
                           kind="ExternalInput").ap()
    ident = nc.dram_tensor("ident", [128, 128], BF16,
                           kind="ExternalInput").ap()
    out = nc.dram_tensor("out", [TL, BPC, H], F32,
                         kind="ExternalOutput").ap()

    exp = mybir.ActivationFunctionType.Exp
    dr = mybir.MatmulPerfMode.DoubleRow

    with tile.TileContext(nc) as tc:
        with (
            tc.tile_pool(name="consts", bufs=1) as consts,
            tc.tile_pool(name="stage_e", bufs=4) as stage_e_pool,
            tc.tile_pool(name="stage_d", bufs=4) as stage_d_pool,
            tc.tile_pool(name="oenat", bufs=2 * NS) as oenat_pool,
            tc.tile_pool(name="oet", bufs=2 * NS) as oet_pool,
            tc.tile_pool(name="odt", bufs=2 * NTC) as odt_pool,
            tc.tile_pool(name="pbuf", bufs=8) as p_pool,
            tc.tile_pool(name="d8buf", bufs=3 * NS // 2) as d8_pool,
            tc.tile_pool(name="oe8buf", bufs=NS) as oe8_pool,
            tc.tile_pool(name="osb", bufs=3) as osb_pool,
            tc.tile_pool(name="small", bufs=4) as small_pool,
            tc.tile_pool(name="psS", bufs=3, space="PSUM") as psS_pool,
            tc.tile_pool(name="psC", bufs=2, space="PSUM") as psC_pool,
            tc.tile_pool(name="psD", bufs=1, space="PSUM") as psD_pool,
            tc.tile_pool(name="ptr", bufs=2, space="PSUM") as ptr_pool,
        ):
            ones = consts.tile([128, 1], BF16, tag="ones")
            nc.vector.memset(ones, 1.0)
            ones8 = consts.tile([128, 2, 1], FP8, tag="ones8")
            nc.vector.memset(ones8, 1.0)
            onesK1 = consts.tile([1, 128], BF16, tag="onesK1")
            nc.vector.memset(onesK1, 1.0)
            # DSCALE * SL for the denominator constant (scales cancel in
            # the final psC * recip(psD))
            denc = consts.tile([1, 1], BF16, tag="denc")
            nc.vector.memset(denc, float(DSCALE * SL))
            idt = consts.tile([128, 128], BF16, tag="idt")
            nc.sync.dma_start(idt, ident)

            # HAM warmup: un-throttle the PE clock before the load phase.
            warm = consts.tile([128, TCHUNK], BF16, tag="warm")
            nc.vector.memset(warm, 0.25)
            wt = ptr_pool.tile([128, TCHUNK], F32, tag="ptr")
            for _ in range(28):
                nc.tensor.matmul(wt, warm[:, 0:128], warm,
                                 start=True, stop=True)

            def transpose_tiles(src, dst):
                """src [128, NH*128] bf16 -> dst [128, NH, 128] fp8 with
                dst[p, c, j] = SCALE * src[j, c*128 + p], via NH identity
                matmuls packed into one PSUM bank + one DVE copy-cast."""
                pt = ptr_pool.tile([128, NH * 128], F32, tag="ptr")
                for c in range(NH):
                    nc.tensor.matmul(pt[:, c * 128:(c + 1) * 128],
                                     src[:, c * 128:(c + 1) * 128], idt,
                                     start=True, stop=True)
                nc.vector.tensor_copy(dst, pt)

            HEAD_CHUNKS = 2
            for b in range(BPC):
                # per-s-tile buffers for this batch
                oe_tiles = []    # [128, H] bf16, natural layout (colsum)
                oe8_pairs = []   # [128, 2, H] fp8: s-tile pairs (mm2 rhs)
                oeT_tiles = []   # [128, NH, 128] fp8, h-major, x SCALE
                odT_chunks = []  # [128, NH, TCHUNK] fp8, h-major, x SCALE
                d8_pairs = {tci: [] for tci in range(NTC)}

                def load_d(ci):
                    # one t-chunk (4 t-tiles) per merged SWDGE cast-load
                    odc = odt_pool.tile([128, NH, TCHUNK], FP8, tag="odT",
                                        name=f"odT_{b}_{ci}")
                    odT_chunks.append(odc)
                    sd = stage_d_pool.tile([128, TPC, H], BF16, tag="sd",
                                           name=f"sd_{b}_{ci}")
                    src = out_d[ci * TCHUNK:(ci + 1) * TCHUNK, b, :]
                    nc.gpsimd.dma_start(
                        sd, src.rearrange("(k p) h -> p k h", p=128))
                    for k in range(TPC):
                        transpose_tiles(sd[:, k, :],
                                        odc[:, :, k * 128:(k + 1) * 128])

                def load_e(j):
                    # two s-tiles (both halves) per merged SWDGE cast-load
                    st = stage_e_pool.tile([128, 2, 2 * H], BF16, tag="st",
                                           name=f"st_{b}_{j}")
                    src = out_e[j * 256:(j + 1) * 256, b, :]
                    nc.gpsimd.dma_start(
                        st, src.rearrange("(k p) h -> p k h", p=128))
                    oe8 = oe8_pool.tile([128, 2, H], FP8, tag="oe8",
                                        name=f"oe8_{b}_{j}")
                    oe8_pairs.append(oe8)
                    for k in range(2):
                        oe = oenat_pool.tile([128, H], BF16, tag="oe",
                                             name=f"oe_{b}_{2 * j + k}")
                        oeT = oet_pool.tile([128, NH, 128], FP8, tag="oeT",
                                            name=f"oeT_{b}_{2 * j + k}")
                        oe_tiles.append(oe)
                        oeT_tiles.append(oeT)
                        nc.vector.tensor_add(oe, st[:, k, 0:H],
                                             st[:, k, H:2 * H])
                        transpose_tiles(oe, oeT)
                        nc.vector.tensor_copy(oe8[:, k, :], oe)

                def mm1(tci, i):
                    psS = psS_pool.tile([128, TCHUNK], F32, tag="psS")
                    for c2 in range(NH // 2):
                        nc.tensor.matmul(
                            psS,
                            oeT_tiles[i][:, 2 * c2:2 * c2 + 2, :],
                            odT_chunks[tci][:, 2 * c2:2 * c2 + 2, :],
                            start=(c2 == 0), stop=(c2 == NH // 2 - 1),
                            perf_mode=dr)
                    P = p_pool.tile([128, TCHUNK], BF16, tag="P",
                                    name=f"P_{b}_{tci}_{i}")
                    # undo the SCALE^2 from the pre-scaled transposes
                    nc.scalar.activation(P, psS, exp,
                                         scale=1.0 / (SCALE * SCALE))
                    # d = DSCALE * (p - 1): fp8-friendly residual for mm2
                    if i % 2 == 0:
                        d8 = d8_pool.tile([128, 2, TCHUNK], FP8, tag="d8",
                                          name=f"d8_{b}_{tci}_{i // 2}")
                        d8_pairs[tci].append(d8)
                    nc.vector.tensor_scalar(d8_pairs[tci][i // 2][:, i % 2, :],
                                            P, -1.0, DSCALE,
                                            mybir.AluOpType.add,
                                            mybir.AluOpType.mult)

                def colsum():
                    # colsum_row[h] = DSCALE * sum_s oe[s, h] (bf16 oe, exact
                    # part of the p = 1 + d decomposition)
                    pcs = ptr_pool.tile([1, H], F32, tag="ptr")
                    for i in range(NS):
                        nc.tensor.matmul(pcs, ones, oe_tiles[i],
                                         start=(i == 0), stop=(i == NS - 1))
                    cs = small_pool.tile([1, H], BF16, tag="cs", bufs=2)
                    nc.vector.tensor_scalar(cs, pcs, DSCALE, None,
                                            mybir.AluOpType.mult)
                    return cs

                def mm2(tci, cs):
                    for tt in range(TPC):
                        psC = psC_pool.tile([128, H], F32, tag="psC")
                        psD = psD_pool.tile([128, 1], F32, tag="psD")
                        # constant terms via K=1 broadcast matmuls:
                        # psC = DSCALE*colsum[h] (for all t), psD = DSCALE*SL
                        nc.tensor.matmul(psC, onesK1, cs,
                                         start=True, stop=False)
                        nc.tensor.matmul(psD, onesK1, denc,
                                         start=True, stop=False)
                        for j in range(NS // 2):
                            lhsT = d8_pairs[tci][j][:, :,
                                                    tt * 128:(tt + 1) * 128]
                            nc.tensor.matmul(psC, lhsT, oe8_pairs[j],
                                             start=False,
                                             stop=(j == NS // 2 - 1),
                                             perf_mode=dr)
                            nc.tensor.matmul(psD, lhsT, ones8,
                                             start=False,
                                             stop=(j == NS // 2 - 1),
                                             perf_mode=dr)
                        rc = small_pool.tile([128, 1], F32, tag="rc")
                        nc.vector.reciprocal(rc, psD)
                        ob = osb_pool.tile([128, H], F32, tag="ob")
                        nc.vector.tensor_scalar(ob, psC, rc, None,
                                                mybir.AluOpType.mult)
                        t0 = tci * TCHUNK + tt * 128
                        nc.sync.dma_start(out[t0:t0 + 128, b, :], ob)

                # Load phase: merged loads stream; transposes follow each
                # arrival; mm1 for the first HEAD_CHUNKS t-chunks trails one
                # e-load behind (hides the PE->DVE->PE oeT round trip).
                for ci in range(HEAD_CHUNKS):
                    load_d(ci)
                for j in range(NS // 2):
                    load_e(j)
                    if j < NTC - HEAD_CHUNKS:
                        load_d(HEAD_CHUNKS + j)
                    if j >= 1:
                        for s in (2 * (j - 1), 2 * j - 1):
                            for tci in range(HEAD_CHUNKS):
                                mm1(tci, s)
                for s in (NS - 2, NS - 1):
                    for tci in range(HEAD_CHUNKS):
                        mm1(tci, s)

                # Steady phase: alternate remaining mm1 chunks with mm2.
                cs = colsum()
                for tci in range(HEAD_CHUNKS, NTC):
                    for i in range(NS):
                        mm1(tci, i)
                    mm2(tci - HEAD_CHUNKS, cs)
                for tci in range(NTC - HEAD_CHUNKS, NTC):
                    mm2(tci, cs)

    nc.compile()
    return nc


_nc = None
last_result = None
_IDENT = (np.eye(128) * SCALE).astype(ml_dtypes.bfloat16)


def kernel(in_e=None, out_e=None, out_d=None, _trace=False, **_unused):
    global _nc, last_result
    if _nc is None:
        _nc = build()
    out_e = np.asarray(out_e, dtype=np.float32)
    out_d = np.asarray(out_d, dtype=np.float32)
    in_maps = []
    for c in range(NCORES):
        sl = slice(c * BPC, (c + 1) * BPC)
        in_maps.append({
            "out_e": np.ascontiguousarray(out_e[:, sl, :]),
            "out_d": np.ascontiguousarray(out_d[:, sl, :]),
            "ident": _IDENT,
        })
    last_result = run_bass_kernel_spmd(_nc, in_maps,
                                       core_ids=list(range(NCORES)),
                                       trace=_trace)
    return np.concatenate(
        [np.asarray(last_result.results[c]["out"]) for c in range(NCORES)],
        axis=1).astype(np.float32)


# revision 23
# speedup vs baseline: 1.3725x; 1.0003x over previous
"""Trainium2 Bass kernel for nn_Attention_62938450756123.

Reference computation (per batch b):
    oe[s, h] = out_e[s, b, 0:512] + out_e[s, b, 512:1024]      # bidirectional sum
    od[t, h] = out_d[t, b, :]
    S[s, t]  = sum_h oe[s, h] * od[t, h]
    p[s, t]  = exp(S[s, t])                                     # naive, no max-sub
    ctx[t,h] = (sum_s p[s, t] * oe[s, h]) / (sum_s p[s, t])
    out[t, b, h] = ctx[t, h]

Sharding: data-parallel over batch (bs=16) across 8 NeuronCores, 2 batches
per core, no collectives.

Per-core dataflow:
  - GPSIMD (SWDGE) cast-loads f32->bf16: out_e halves + out_d tiles.
  - VectorE sums the out_e halves -> oe tiles bf16 [s128, h512] (mm2 rhs).
  - h-major layouts for mm1 are built ON TensorE: for each 128x128 block,
    psum[h, s'] = sum_s x[s, h] * (SCALE * I[s, s'])  (normal matmul,
    scaled identity moving, ~56ns warm).  Four h-chunks pack into one PSUM
    bank; one VectorE copy casts the bank to fp8e4m3 SBUF:
    oeT_i [128p, 4hc, 128s], odT_chunk [128p, 4hc, 512t], h = hc*128 + p,
    values pre-scaled by SCALE=32 to sit in fp8's normal range.
    (DMA-xbar transposes are NOT used: Tile serializes them against every
    other DMA - HW-deadlock workaround - which ping-pongs the load stream.)
  - mm1 runs in fp8 with perf_mode=DoubleRow (2 fp8 weights/PE cell):
    psum_S[s128, t512] accumulates over 2 k-tiles of [128p x 2ko] = 256,
    at ~2x bf16 matmul rate.  ScalarE exp applies scale=1/SCALE^2 to undo
    the identity pre-scaling: P = exp(S_psum / 1024) in bf16.
    Softmax output error stays ~1e-3: S absolute error ~1e-3 from fp8
    inputs, and the near-uniform softmax averages 2048 terms.
  - mm2 stays bf16: psum_ctx[t128, h512] += P_i.T @ oe_i ;
    psum_den[t128, 1] += P_i.T @ ones (same stationary weights, +25ns/pair)
  - normalize on VectorE (reciprocal + tensor_scalar), store via Sync HWDGE.
  - ~6us dummy-matmul warmup un-throttles the HAM PE clock gate before the
    load phase; chunk-0 mm1 trails the per-tile transposes by one s-tile.

Buffers are allocated per-s-tile (separate Tile objects) so dependency
tracking stays precise.
"""

import ml_dtypes
import numpy as np

import concourse.bass as bass
import concourse.tile as tile
from concourse import bacc, mybir
from concourse.bass_utils import run_bass_kernel_spmd

SL, TL, BS, H = 2048, 2048, 16, 512
NCORES = 8
BPC = BS // NCORES  # batches per core

F32 = mybir.dt.float32
BF16 = mybir.dt.bfloat16
FP8 = mybir.dt.float8e4

NS = SL // 128        # 16 s-tiles
NH = H // 128         # 4 h-chunks
TCHUNK = 512          # t-chunk (one PSUM bank of f32)
NTC = TL // TCHUNK    # 4 t-chunks
TPC = TCHUNK // 128   # 4 t-tiles per chunk
SCALE = 32.0          # fp8 pre-scale (folded into the transpose identity)
DSCALE = 16.0         # fp8 pre-scale for d = p - 1 (|d| <~ 0.06 -> ~1)


def build():
    nc = bacc.Bacc("TRN2", target_bir_lowering=False, debug=False,
                   num_devices=NCORES)
    out_e = nc.dram_tensor("out_e", [SL, BPC, 2 * H], F32,
                           kind="ExternalInput").ap()
    out_d = nc.dram_tensor("out_d", [TL, BPC, H], F32,
                           kind="ExternalInput").ap()
    ident = nc.dram_tensor("ident", [128, 128], BF16,
                           kind="ExternalInput").ap()
    out = nc.dram_tensor("out", [TL, BPC, H], F32,
                         kind="ExternalOutput").ap()

    exp = mybir.ActivationFunctionType.Exp
    dr = mybir.MatmulPerfMode.DoubleRow

    with tile.TileContext(nc) as tc:
        with (
            tc.tile_pool(name="consts", bufs=1) as consts,
            tc.tile_pool(name="stage_e", bufs=4) as stage_e_pool,
            tc.tile_pool(name="stage_d", bufs=4) as stage_d_pool,
            tc.tile_pool(name="oenat", bufs=2 * NS) as oenat_pool,
            tc.tile_pool(name="oet", bufs=2 * NS) as oet_pool,
            tc.tile_pool(name="odt", bufs=2 * NTC) as odt_pool,
            tc.tile_pool(name="pbuf", bufs=8) as p_pool,
            tc.tile_pool(name="d8buf", bufs=3 * NS // 2) as d8_pool,
            tc.tile_pool(name="oe8buf", bufs=NS) as oe8_pool,
            tc.tile_pool(name="osb", bufs=3) as osb_pool,
            tc.tile_pool(name="small", bufs=4) as small_pool,
            tc.tile_pool(name="psS", bufs=3, space="PSUM") as psS_pool,
            tc.tile_pool(name="psC", bufs=2, space="PSUM") as psC_pool,
            tc.tile_pool(name="psD", bufs=1, space="PSUM") as psD_pool,
            tc.tile_pool(name="ptr", bufs=2, space="PSUM") as ptr_pool,
        ):
            ones = consts.tile([128, 1], BF16, tag="ones")
            nc.vector.memset(ones, 1.0)
            ones8 = consts.tile([128, 2, 1], FP8, tag="ones8")
            nc.vector.memset(ones8, 1.0)
            onesK1 = consts.tile([1, 128], BF16, tag="onesK1")
            nc.vector.memset(onesK1, 1.0)
            # DSCALE * SL for the denominator constant (scales cancel in
            # the final psC * recip(psD))
            denc = consts.tile([1, 1], BF16, tag="denc")
            nc.vector.memset(denc, float(DSCALE * SL))
            idt = consts.tile([128, 128], BF16, tag="idt")
            nc.sync.dma_start(idt, ident)

            # HAM warmup: un-throttle the PE clock before the load phase.
            warm = consts.tile([128, TCHUNK], BF16, tag="warm")
            nc.vector.memset(warm, 0.25)
            wt = ptr_pool.tile([128, TCHUNK], F32, tag="ptr")
            for _ in range(28):
                nc.tensor.matmul(wt, warm[:, 0:128], warm,
                                 start=True, stop=True)

            def transpose_tiles(src, dst):
                """src [128, NH*128] bf16 -> dst [128, NH, 128] fp8 with
                dst[p, c, j] = SCALE * src[j, c*128 + p], via NH identity
                matmuls packed into one PSUM bank + one DVE copy-cast."""
                pt = ptr_pool.tile([128, NH * 128], F32, tag="ptr")
                for c in range(NH):
                    nc.tensor.matmul(pt[:, c * 128:(c + 1) * 128],
                                     src[:, c * 128:(c + 1) * 128], idt,
                                     start=True, stop=True)
                nc.vector.tensor_copy(dst, pt)

            HEAD_CHUNKS = 2
            for b in range(BPC):
                # per-s-tile buffers for this batch
                oe_tiles = []    # [128, H] bf16, natural layout (colsum)
                oe8_pairs = []   # [128, 2, H] fp8: s-tile pairs (mm2 rhs)
                oeT_tiles = []   # [128, NH, 128] fp8, h-major, x SCALE
                odT_chunks = []  # [128, NH, TCHUNK] fp8, h-major, x SCALE
                d8_pairs = {tci: [] for tci in range(NTC)}

                def load_d(ci):
                    # one t-chunk (4 t-tiles) per merged SWDGE cast-load
                    odc = odt_pool.tile([128, NH, TCHUNK], FP8, tag="odT",
                                        name=f"odT_{b}_{ci}")
                    odT_chunks.append(odc)
                    sd = stage_d_pool.tile([128, TPC, H], BF16, tag="sd",
                                           name=f"sd_{b}_{ci}")
                    src = out_d[ci * TCHUNK:(ci + 1) * TCHUNK, b, :]
                    nc.gpsimd.dma_start(
                        sd, src.rearrange("(k p) h -> p k h", p=128))
                    for k in range(TPC):
                        transpose_tiles(sd[:, k, :],
                                        odc[:, :, k * 128:(k + 1) * 128])

                def load_e(j):
                    # two s-tiles (both halves) per merged SWDGE cast-load
                    st = stage_e_pool.tile([128, 2, 2 * H], BF16, tag="st",
                                           name=f"st_{b}_{j}")
                    src = out_e[j * 256:(j + 1) * 256, b, :]
                    nc.gpsimd.dma_start(
                        st, src.rearrange("(k p) h -> p k h", p=128))
                    oe8 = oe8_pool.tile([128, 2, H], FP8, tag="oe8",
                                        name=f"oe8_{b}_{j}")
                    oe8_pairs.append(oe8)
                    for k in range(2):
                        oe = oenat_pool.tile([128, H], BF16, tag="oe",
                                             name=f"oe_{b}_{2 * j + k}")
                        oeT = oet_pool.tile([128, NH, 128], FP8, tag="oeT",
                                            name=f"oeT_{b}_{2 * j + k}")
                        oe_tiles.append(oe)
                        oeT_tiles.append(oeT)
                        nc.vector.tensor_add(oe, st[:, k, 0:H],
                                             st[:, k, H:2 * H])
                        transpose_tiles(oe, oeT)
                        nc.vector.tensor_copy(oe8[:, k, :], oe)

                def mm1(tci, i):
                    psS = psS_pool.tile([128, TCHUNK], F32, tag="psS")
                    for c2 in range(NH // 2):
                        nc.tensor.matmul(
                            psS,
                            oeT_tiles[i][:, 2 * c2:2 * c2 + 2, :],
                            odT_chunks[tci][:, 2 * c2:2 * c2 + 2, :],
                            start=(c2 == 0), stop=(c2 == NH // 2 - 1),
                            perf_mode=dr)
                    P = p_pool.tile([128, TCHUNK], BF16, tag="P",
                                    name=f"P_{b}_{tci}_{i}")
                    # undo the SCALE^2 from the pre-scaled transposes
                    nc.scalar.activation(P, psS, exp,
                                         scale=1.0 / (SCALE * SCALE))
                    # d = DSCALE * (p - 1): fp8-friendly residual for mm2
                    if i % 2 == 0:
                        d8 = d8_pool.tile([128, 2, TCHUNK], FP8, tag="d8",
                                          name=f"d8_{b}_{tci}_{i // 2}")
                        d8_pairs[tci].append(d8)
                    nc.vector.tensor_scalar(d8_pairs[tci][i // 2][:, i % 2, :],
                                            P, -1.0, DSCALE,
                                            mybir.AluOpType.add,
                                            mybir.AluOpType.mult)

                def colsum():
                    # colsum_row[h] = DSCALE * sum_s oe[s, h] (bf16 oe, exact
                    # part of the p = 1 + d decomposition)
                    pcs = ptr_pool.tile([1, H], F32, tag="ptr")
                    for i in range(NS):
                        nc.tensor.matmul(pcs, ones, oe_tiles[i],
                                         start=(i == 0), stop=(i == NS - 1))
                    cs = small_pool.tile([1, H], BF16, tag="cs", bufs=2)
                    nc.vector.tensor_scalar(cs, pcs, DSCALE, None,
                                            mybir.AluOpType.mult)
                    return cs

                def mm2(tci, cs, feed=None):
                    # feed: optional iterator of thunks (mm1 emissions for a
                    # later chunk), interleaved one per two DR pair-slots so
                    # the PE has exp-independent work while ScalarE catches
                    # up on the exponentials.
                    slot = 0
                    for tt in range(TPC):
                        psC = psC_pool.tile([128, H], F32, tag="psC")
                        psD = psD_pool.tile([128, 1], F32, tag="psD")
                        # constant terms via K=1 broadcast matmuls:
                        # psC = DSCALE*colsum[h] (for all t), psD = DSCALE*SL
                        nc.tensor.matmul(psC, onesK1, cs,
                                         start=True, stop=False)
                        nc.tensor.matmul(psD, onesK1, denc,
                                         start=True, stop=False)
                        for j in range(NS // 2):
                            if feed is not None and slot % 2 == 0:
                                thunk = next(feed, None)
                                if thunk is not None:
                                    thunk()
                            slot += 1
                            lhsT = d8_pairs[tci][j][:, :,
                                                    tt * 128:(tt + 1) * 128]
                            nc.tensor.matmul(psC, lhsT, oe8_pairs[j],
                                             start=False,
                                             stop=(j == NS // 2 - 1),
                                             perf_mode=dr)
                            nc.tensor.matmul(psD, lhsT, ones8,
                                             start=False,
                                             stop=(j == NS // 2 - 1),
                                             perf_mode=dr)
                        rc = small_pool.tile([128, 1], F32, tag="rc")
                        nc.vector.reciprocal(rc, psD)
                        ob = osb_pool.tile([128, H], F32, tag="ob")
                        nc.vector.tensor_scalar(ob, psC, rc, None,
                                                mybir.AluOpType.mult)
                        t0 = tci * TCHUNK + tt * 128
                        nc.sync.dma_start(out[t0:t0 + 128, b, :], ob)

                # Load phase: merged loads stream; transposes follow each
                # arrival; mm1 for the first HEAD_CHUNKS t-chunks trails one
                # e-load behind (hides the PE->DVE->PE oeT round trip).
                for ci in range(HEAD_CHUNKS):
                    load_d(ci)
                for j in range(NS // 2):
                    load_e(j)
                    if j < NTC - HEAD_CHUNKS:
                        load_d(HEAD_CHUNKS + j)
                    if j >= 1:
                        for s in (2 * (j - 1), 2 * j - 1):
                            for tci in range(HEAD_CHUNKS):
                                mm1(tci, s)
                for s in (NS - 2, NS - 1):
                    for tci in range(HEAD_CHUNKS):
                        mm1(tci, s)

                # Steady phase: interleave remaining mm1 chunks into mm2.
                cs = colsum()
                for tci in range(HEAD_CHUNKS, NTC):
                    feed = iter([(lambda i=i, t=tci: mm1(t, i))
                                 for i in range(NS)])
                    mm2(tci - HEAD_CHUNKS, cs, feed)
                for tci in range(NTC - HEAD_CHUNKS, NTC):
                    mm2(tci, cs)

    nc.compile()
    return nc


_nc = None
last_result = None
_IDENT = (np.eye(128) * SCALE).astype(ml_dtypes.bfloat16)


def kernel(in_e=None, out_e=None, out_d=None, _trace=False, **_unused):
    global _nc, last_result
    if _nc is None:
        _nc = build()
    out_e = np.asarray(out_e, dtype=np.float32)
    out_d = np.asarray(out_d, dtype=np.float32)
    in_maps = []
    for c in range(NCORES):
        sl = slice(c * BPC, (c + 1) * BPC)
        in_maps.append({
            "out_e": np.ascontiguousarray(out_e[:, sl, :]),
            "out_d": np.ascontiguousarray(out_d[:, sl, :]),
            "ident": _IDENT,
        })
    last_result = run_bass_kernel_spmd(_nc, in_maps,
                                       core_ids=list(range(NCORES)),
                                       trace=_trace)
    return np.concatenate(
        [np.asarray(last_result.results[c]["out"]) for c in range(NCORES)],
        axis=1).astype(np.float32)
